# revision 20
# baseline (speedup 1.0000x reference)
"""Trainium2 Bass kernel for nn_ExpertsLinearEnsemble.

Reference computation (B=16384, D=768, E=124, C=6):
  expert_logits  = Mlp_cls(x).reshape(B, E, C)     # D -> D -> gelu -> E*C
  ew_logits      = Mlp_ew(x)                       # D -> D -> gelu -> E
  which_expert   = Mlp_we(x)                       # D -> D -> gelu -> E
  n = clamp(n_experts, E); thr = n-th largest of which_expert per row
  mask out experts with which_expert < thr; softmax ew_logits over kept
  experts; softmax expert_logits over classes; combined = sum_e w_e *
  proba_e / sum_e w_e.

Strategy (pure data parallel, 2048 rows/core):
  - Host transposes x so the contraction dim D sits on SBUF partitions;
    the whole device pipeline runs feature-major ([feature, row] tiles).
  - Precision split, driven by a Monte-Carlo study of the top-n mask:
    the which_expert logits must stay within ~3e-7 of the reference's
    fp32 values (the n-th/(n+1)-th gap distribution has a heavy tail:
    ~0.1% of rows sit closer than 5e-6), while ew tolerates ~1e-2 and
    cls ~1e-3.  So:
      * we layer 1 runs as a 3-term fp16 hi/lo split (xh@wh + xh@wl +
        xl@wh, each at the full 1 cycle/row rate) giving ~1.5e-7 rms at
        73% of the fp32 matmul cost; layer 2 stays true fp32.
      * cls runs in fp16 (same speed as bf16, 30x the accuracy).
      * ew runs in fp8e4m3 with DoubleRow pairing (2 k-tiles per
        instruction, 2x throughput).
  - cls layer-2 columns are permuted to class-major (tile t2 = class
    t2), so the per-expert class softmax sum is 5 vector adds and the
    expert-weight broadcast needs no PE replication matmuls.
  - Top-n threshold per row: rows are sorted by n on host and dealt
    round-robin to cores, so each 128-row subtile has a narrow n-range
    and a fixed number of max8/match_replace rounds (descending sort for
    small n, ascending for large n) suffices.  The threshold is the
    (n-1)-th sorted value, extracted with a one-hot dot against a host
    supplied selector index; mask = which_expert >= thr matches the
    reference's `which_expert < thr` masking exactly, ties included.
    All sort/compare values stay in exact fp32 (PE transposes are exact
    permutations).
"""

import os
import sys

for _p in ("/opt/trn_rl_repo", "/root/.axon_site/_ro/trn_rl_repo"):
    if os.path.isdir(_p) and _p not in sys.path:
        sys.path.insert(0, _p)

import numpy as np

B, D, E, C = 16384, 768, 124, 6
EC = E * C            # 744
NCORES = 8
BC = B // NCORES      # 2048 rows per core
RT = 512              # rows per macro tile (PSUM bank = 512 fp32)
NT = BC // RT         # 4 macro tiles per core
SUB = 128             # rows per sort subtile
NS = BC // SUB        # 16 subtiles per core
KT = D // 128         # 6 contraction tiles
MT1 = D // 128        # 6 output tiles for layer 1
MT2 = C               # 6 class-major output tiles for the cls head

# Fixed per-subtile sort schedule.  Subtile s holds rows whose global
# sorted-n positions are [1024 s, 1024 (s+1)); with n ~ U[1,124] the
# boundary quantiles sit many sigma inside these capabilities.
# s in [0, 8): descending sort, handles n <= 8 R; s in [8, 16): ascending
# (sort of -we), handles n >= 125 - 8 R.
R_DESC = [2, 3, 4, 5, 6, 7, 8, 9]
R_ASC = [9, 8, 7, 6, 5, 4, 3, 2]
SUB_DIR = [True] * 8 + [False] * 8         # True = descending
SUB_R = R_DESC + R_ASC
FALLBACK_R = [16] * NS                     # safe for any n distribution
FALLBACK_DIR = [True] * NS

NEG_FILL = -1.0e30

_BUILD_CACHE = {}


def _build_nc(sub_dir, sub_r, act="Gelu"):
    """Build the (SPMD, per-core) Bass program.  Data independent."""
    from contextlib import ExitStack

    import concourse.mybir as mybir
    import concourse.tile as tile
    from concourse import bacc

    dt = mybir.dt
    AF = mybir.ActivationFunctionType
    OP = mybir.AluOpType
    DR = mybir.MatmulPerfMode.DoubleRow
    f32 = dt.float32
    f16 = dt.float16
    f8 = dt.float8e4

    nc = bacc.Bacc(
        "TRN2",
        target_bir_lowering=False,
        debug=False,
        enable_asserts=False,
        num_devices=NCORES,
    )

    def din(name, shape, dtype=f32):
        return nc.dram_tensor(name, list(shape), dtype, kind="ExternalInput")

    xh_d = din("xh", [D, BC], f16)              # fp16 hi of x.T
    xl_d = din("xl", [D, BC], f16)              # fp16 lo of x.T
    x8_d = din("x8", [D, BC], f8)               # fp8 x.T (ew path)
    ksel_d = din("ksel", [SUB, NS])
    w1c_d = din("w1c", [D, D], f16)
    w1e_d = din("w1e", [D, D], f8)
    w1wh_d = din("w1wh", [D, D], f16)           # fp16 hi of we_w1
    w1wl_d = din("w1wl", [D, D], f16)           # fp16 lo of we_w1
    b1_d = {m: din(f"b1{m}", [128, MT1]) for m in "cwe"}
    w2c_d = din("w2c", [D, MT2 * 128], f16)     # class-major, padded 124->128
    w2e_d = din("w2e", [D, 128], f8)            # padded 124->128
    w2w_d = din("w2w", [D, E])                  # fp32
    b2c_d = din("b2c", [E, MT2])
    b2w_d = din("b2w", [E, 1])
    b2e_d = din("b2e", [E, 1])
    colsel_d = din("colsel", [E, MT2, C], f16)  # out[c] += sum_e wexp[e,c]
    ident_d = din("ident", [128, 128])
    iota_d = din("iota", [128, 128])
    out_d = nc.dram_tensor("out", [C, BC], f32, kind="ExternalOutput")

    with tile.TileContext(nc) as tc, ExitStack() as ctx:
        const = ctx.enter_context(tc.tile_pool(name="const", bufs=1))
        xtp = ctx.enter_context(tc.tile_pool(name="xtp", bufs=2))
        hp = ctx.enter_context(tc.tile_pool(name="hp", bufs=2))
        epp = ctx.enter_context(tc.tile_pool(name="epp", bufs=2))
        wep = ctx.enter_context(tc.tile_pool(name="wep", bufs=2))
        sp = ctx.enter_context(tc.tile_pool(name="sp", bufs=2))
        wp = ctx.enter_context(tc.tile_pool(name="wp", bufs=2))
        psmm = ctx.enter_context(tc.tile_pool(name="psmm", bufs=3, space="PSUM"))
        pstr = ctx.enter_context(tc.tile_pool(name="pstr", bufs=2, space="PSUM"))
        psmask = ctx.enter_context(tc.tile_pool(name="psmask", bufs=1, space="PSUM"))
        pss = ctx.enter_context(tc.tile_pool(name="pss", bufs=1, space="PSUM"))
        psout = ctx.enter_context(tc.tile_pool(name="psout", bufs=1, space="PSUM"))

        # ---- resident constants / weights -------------------------------
        # Weights ride the gpsimd (SWDGE) queues so the per-tile x DMAs on
        # the sync (HWDGE) queues are not stuck behind ~7 MB of weights;
        # split per k-tile so the first matmul only waits on its own slice.
        def load_w(dram, cols, dtype, tag, eng=None):
            t = const.tile([128, KT, cols], dtype, tag=tag)
            ap = dram.ap().rearrange("(ko p) m -> p ko m", p=128)
            for k in range(KT):
                (eng or nc.gpsimd).dma_start(t[:, k, :], ap[:, k, :])
            return t

        def load_c(dram, shape, dtype, tag):
            t = const.tile(shape, dtype, tag=tag)
            nc.gpsimd.dma_start(t[:], dram.ap())
            return t

        # First macro tile's x slices are split per k-tile on the sync
        # (HWDGE) queue so the first matmul waits only for its own 128 KB;
        # the critical we-path weights head the gpsimd (SWDGE) queue.
        xh_ap = xh_d.ap().rearrange("(ko p) n -> p ko n", p=128)
        xl_ap = xl_d.ap().rearrange("(ko p) n -> p ko n", p=128)
        x8_ap = x8_d.ap().rearrange("(ko p) n -> p ko n", p=128)
        xh0 = xtp.tile([128, KT, RT], f16, tag="xh")
        for k in range(KT):
            nc.sync.dma_start(xh0[:, k, :], xh_ap[:, k, 0:RT])
        xl0 = xtp.tile([128, KT, RT], f16, tag="xl")
        for k in range(KT):
            nc.sync.dma_start(xl0[:, k, :], xl_ap[:, k, 0:RT])
        x80 = xtp.tile([128, KT, RT], f8, tag="x8")
        nc.sync.dma_start(x80[:], x8_ap[:, :, 0:RT])
        w1wh = load_w(w1wh_d, D, f16, "w1wh")
        w1wl = load_w(w1wl_d, D, f16, "w1wl")
        w1c = load_w(w1c_d, D, f16, "w1c")
        w1e = load_w(w1e_d, D, f8, "w1e")
        b1sb = {m: load_c(b1_d[m], [128, MT1], f32, f"b1{m}") for m in "cwe"}
        w2c = load_w(w2c_d, MT2 * 128, f16, "w2c")
        w2w = load_w(w2w_d, E, f32, "w2w")
        w2e = load_w(w2e_d, 128, f8, "w2e")
        b2csb = load_c(b2c_d, [E, MT2], f32, "b2c")
        b2wsb = load_c(b2w_d, [E, 1], f32, "b2w")
        b2esb = load_c(b2e_d, [E, 1], f32, "b2e")
        colsel = load_c(colsel_d, [E, MT2, C], f16, "colsel")
        ident = load_c(ident_d, [128, 128], f32, "ident")
        iotam = load_c(iota_d, [128, 128], f32, "iota")
        kselsb = load_c(ksel_d, [SUB, NS], f32, "ksel")
        ones6 = const.tile([1, C], f16, tag="ones6")
        nc.vector.memset(ones6[:], 1.0)
        ones124 = const.tile([E, 1], f16, tag="ones124")
        nc.vector.memset(ones124[:], 1.0)
        outacc = const.tile([C, BC], f32, tag="outacc")

        for T in range(NT):
            rs = slice(T * RT, (T + 1) * RT)
            if T == 0:
                xh, xl, x8 = xh0, xl0, x80
            else:
                xh = xtp.tile([128, KT, RT], f16, tag="xh")
                nc.sync.dma_start(xh[:], xh_ap[:, :, rs])
                xl = xtp.tile([128, KT, RT], f16, tag="xl")
                nc.sync.dma_start(xl[:], xl_ap[:, :, rs])
                x8 = xtp.tile([128, KT, RT], f8, tag="x8")
                nc.sync.dma_start(x8[:], x8_ap[:, :, rs])

            # ---- which_expert MLP first: its fp32-exact logits gate the
            # (vector-engine) sort, which then overlaps the cls/ew MLPs.
            htw = hp.tile([128, KT, RT], f32, tag="htw")
            for mt in range(MT1):
                ps = psmm.tile([128, RT], f32, tag="psmm")
                cs = slice(mt * 128, (mt + 1) * 128)
                i = 0
                for wt, xt in ((w1wh, xh), (w1wl, xh), (w1wh, xl)):
                    for k in range(KT):
                        nc.tensor.matmul(
                            ps[:], wt[:, k, cs], xt[:, k, :],
                            start=(i == 0), stop=(i == 3 * KT - 1),
                        )
                        i += 1
                nc.scalar.activation(
                    htw[:, mt, :], ps[:], getattr(AF, act),
                    bias=b1sb["w"][:, mt : mt + 1], scale=1.0 / 64.0,
                )
            weT = wep.tile([E, RT], f32, tag="weT")
            ps = psmm.tile([128, RT], f32, tag="psmm")
            for k in range(KT):
                nc.tensor.matmul(
                    ps[:E], w2w[:, k, :], htw[:, k, :],
                    start=(k == 0), stop=(k == KT - 1),
                )
            nc.scalar.activation(weT[:], ps[:E], AF.Identity, bias=b2wsb[:])

            # ---- per-row top-n mask (row-major subtiles) ----------------
            maskT = psmask.tile([E, RT], f32, tag="maskT")
            for j in range(RT // SUB):
                s = (RT // SUB) * T + j
                Rr, desc = sub_r[s], sub_dir[s]
                cs = slice(j * SUB, (j + 1) * SUB)
                trp = pstr.tile([128, 128], f32, tag="trp")
                nc.tensor.transpose(trp[:, :E], weT[:, cs], ident[:E, :E])
                weRow = sp.tile([128, E], f32, tag="weRow")
                nc.scalar.copy(weRow[:], trp[:, :E])
                scratch = sp.tile([128, E], f32, tag="scratch")
                if desc:
                    nc.vector.tensor_copy(scratch[:], weRow[:])
                else:
                    nc.vector.tensor_scalar_mul(scratch[:], weRow[:], -1.0)
                srt = sp.tile([128, 128], f32, tag="srt")
                for r in range(Rr):
                    nc.vector.max(out=srt[:, 8 * r : 8 * r + 8], in_=scratch[:])
                    if r < Rr - 1:
                        nc.vector.match_replace(
                            out=scratch[:],
                            in_to_replace=srt[:, 8 * r : 8 * r + 8],
                            in_values=scratch[:],
                            imm_value=NEG_FILL,
                        )
                w8 = 8 * Rr
                ohtmp = sp.tile([128, 128], f32, tag="ohtmp")
                thr = sp.tile([128, 1], f32, tag="thr")
                nc.vector.scalar_tensor_tensor(
                    out=ohtmp[:, :w8],
                    in0=iotam[:, :w8],
                    scalar=kselsb[:, s : s + 1],
                    in1=srt[:, :w8],
                    op0=OP.is_equal,
                    op1=OP.mult,
                    accum_out=thr[:],
                )
                if not desc:
                    nc.vector.tensor_scalar_mul(thr[:], thr[:], -1.0)
                maskRow = sp.tile([128, E], f32, tag="maskRow")
                nc.vector.tensor_scalar(maskRow[:], weRow[:], thr[:], None, OP.is_ge)
                nc.tensor.transpose(maskT[:, cs], maskRow[:], ident[:])

            # ---- cls + ew MLPs (overlap the sort above) -----------------
            htc = hp.tile([128, KT, RT], f16, tag="htc")
            for mt in range(MT1):
                ps = psmm.tile([128, RT], f32, tag="psmm")
                cs = slice(mt * 128, (mt + 1) * 128)
                for k in range(KT):
                    nc.tensor.matmul(
                        ps[:], w1c[:, k, cs], xh[:, k, :],
                        start=(k == 0), stop=(k == KT - 1),
                    )
                nc.scalar.activation(
                    htc[:, mt, :], ps[:], getattr(AF, act),
                    bias=b1sb["c"][:, mt : mt + 1],
                )
            hte = hp.tile([128, KT, RT], f8, tag="hte")
            for mt in range(MT1):
                ps = psmm.tile([128, RT], f32, tag="psmm")
                cs = slice(mt * 128, (mt + 1) * 128)
                for k2 in range(KT // 2):
                    nc.tensor.matmul(
                        ps[:], w1e[:, 2 * k2 : 2 * k2 + 2, cs],
                        x8[:, 2 * k2 : 2 * k2 + 2, :],
                        start=(k2 == 0), stop=(k2 == KT // 2 - 1),
                        perf_mode=DR,
                    )
                nc.scalar.activation(
                    hte[:, mt, :], ps[:], getattr(AF, act),
                    bias=b1sb["e"][:, mt : mt + 1], scale=1.0 / 128.0,
                )
            expP = epp.tile([E, MT2, RT], f16, tag="expP")
            for t2 in range(MT2):
                ps = psmm.tile([128, RT], f32, tag="psmm")
                cs = slice(t2 * 128, (t2 + 1) * 128)
                for k in range(KT):
                    nc.tensor.matmul(
                        ps[:], w2c[:, k, cs], htc[:, k, :],
                        start=(k == 0), stop=(k == KT - 1),
                    )
                nc.scalar.activation(
                    expP[:, t2, :], ps[:E], AF.Exp, bias=b2csb[:, t2 : t2 + 1]
                )
            expew = wep.tile([E, RT], f16, tag="expew")
            ps = psmm.tile([128, RT], f32, tag="psmm")
            for k2 in range(KT // 2):
                nc.tensor.matmul(
                    ps[:], w2e[:, 2 * k2 : 2 * k2 + 2, :],
                    hte[:, 2 * k2 : 2 * k2 + 2, :],
                    start=(k2 == 0), stop=(k2 == KT // 2 - 1),
                    perf_mode=DR,
                )
            nc.scalar.activation(expew[:], ps[:E], AF.Exp, bias=b2esb[:],
                                 scale=1.0 / 32.0)

            # ---- combine ------------------------------------------------
            # S[e,r] = sum_c expP[e,c,r]  (class softmax denominator)
            S_sb = wp.tile([E, RT], f32, tag="S_sb")
            nc.vector.tensor_tensor(S_sb[:], expP[:, 0, :], expP[:, 1, :], OP.add)
            for t2 in range(2, MT2):
                nc.vector.tensor_tensor(S_sb[:], S_sb[:], expP[:, t2, :], OP.add)
            wT = wp.tile([E, RT], f16, tag="wT")
            nc.vector.tensor_tensor(wT[:], expew[:], maskT[:], OP.mult)
            den_ps = pss.tile([E, RT], f32, tag="S")
            nc.tensor.matmul(den_ps[:1, :], ones124[:], wT[:], start=True, stop=True)
            Sr = wp.tile([E, RT], f32, tag="Sr")
            nc.vector.reciprocal_approx_fast(Sr[:], S_sb[:])
            u = wp.tile([E, RT], f16, tag="u")
            nc.vector.tensor_tensor(u[:], wT[:], Sr[:], OP.mult)
            out_ps = psout.tile([C, RT], f32, tag="out")
            for t2 in range(MT2):
                wexp = wp.tile([E, RT], f16, tag="wexp")
                nc.vector.tensor_tensor(wexp[:], expP[:, t2, :], u[:], OP.mult)
                nc.tensor.matmul(
                    out_ps[:],
                    colsel[:, t2, :],
                    wexp[:],
                    start=(t2 == 0),
                    stop=(t2 == MT2 - 1),
                )
            # normalize by the expert-weight sum and ship this tile's rows
            den_sb = wp.tile([1, RT], f16, tag="den_sb")
            nc.scalar.copy(den_sb[:], den_ps[:1, :])
            rep = pss.tile([E, RT], f32, tag="S")
            nc.tensor.matmul(rep[:C, :], ones6[:], den_sb[:], start=True, stop=True)
            recipd = wp.tile([C, RT], f32, tag="recipd")
            nc.vector.reciprocal_approx_fast(recipd[:], rep[:C, :])
            nc.vector.tensor_tensor(outacc[:, rs], out_ps[:], recipd[:], OP.mult)
            nc.sync.dma_start(out_d.ap()[:, rs], outacc[:, rs])

    nc.compile()
    return nc


def _get_nc(sub_dir, sub_r, act="Gelu"):
    key = (tuple(sub_dir), tuple(sub_r), act)
    if key not in _BUILD_CACHE:
        _BUILD_CACHE[key] = _build_nc(sub_dir, sub_r, act)
    return _BUILD_CACHE[key]


def _host_prep(x, n_experts):
    n = np.minimum(np.asarray(n_experts).astype(np.int64), E).astype(np.int32)
    order = np.argsort(n, kind="stable")
    ns_sorted = n[order]

    sub_dir, sub_r = SUB_DIR, SUB_R
    ok = True
    for s in range(NS):
        lo = int(ns_sorted[(B // NS) * s])
        hi = int(ns_sorted[(B // NS) * (s + 1) - 1])
        if sub_dir[s]:
            ok &= hi <= 8 * sub_r[s]
        else:
            ok &= lo >= E + 1 - 8 * sub_r[s]
    if not ok:
        sub_dir, sub_r = FALLBACK_DIR, FALLBACK_R

    rows_by_core = [order[c::NCORES] for c in range(NCORES)]
    xts, ksels = [], []
    for c in range(NCORES):
        rows = rows_by_core[c]
        xts.append(np.ascontiguousarray(x[rows].T.astype(np.float32)))
        nv = n[rows].astype(np.float32)
        ks = np.empty(BC, np.float32)
        for s in range(NS):
            seg = slice(SUB * s, SUB * (s + 1))
            ks[seg] = (nv[seg] - 1.0) if sub_dir[s] else (E - nv[seg])
        ksels.append(np.ascontiguousarray(ks.reshape(NS, SUB).T))
    return rows_by_core, xts, ksels, sub_dir, sub_r


def _host_consts():
    colsel = np.zeros((E, MT2, C), np.float32)
    for t in range(MT2):
        colsel[:, t, t] = 1.0
    ident = np.eye(128, dtype=np.float32)
    iota = np.broadcast_to(np.arange(128, dtype=np.float32), (128, 128)).copy()
    return colsel, ident, iota


def _host_inputs(inputs):
    """All DRAM input arrays except the per-core xh/xl/x8/ksel."""
    import ml_dtypes

    f16 = np.float16
    f8 = ml_dtypes.float8_e4m3
    colsel, ident, iota = _host_consts()
    f32 = np.float32

    # x64 pre-scale keeps the lo-residual weights inside fp16's normal
    # range (raw residuals ~5e-6 are deep subnormal at only ~7 bits);
    # the layer-1 activation un-scales with scale=1/64 (exact).
    we_w1 = np.asarray(inputs["we_w1"], f32) * 64.0
    w1wh = we_w1.astype(f16)
    w1wl = (we_w1 - w1wh.astype(f32)).astype(f16)

    # cls layer 2: class-major permute + pad 124 -> 128 per class tile.
    cls_w2 = np.asarray(inputs["cls_w2"], f32)       # [D, E*C], f = e*C + c
    cls_b2 = np.asarray(inputs["cls_b2"], f32)
    w2c = np.zeros((D, MT2 * 128), f32)
    b2c = np.empty((E, MT2), f32)
    for c in range(C):
        w2c[:, c * 128 : c * 128 + E] = cls_w2[:, np.arange(E) * C + c]
        b2c[:, c] = cls_b2[np.arange(E) * C + c]

    # fp8 e4m3 min normal is 2^-6; x32 scale lifts the ~0.02-scale ew
    # weights out of the subnormal range (Exp activation scale=1/32).
    w2e = np.zeros((D, 128), f32)
    w2e[:, :E] = np.asarray(inputs["ew_w2"], f32) * 32.0

    return {
        "w1c": np.asarray(inputs["cls_w1"], f32).astype(f16),
        "w1wh": w1wh,
        "w1wl": w1wl,
        "w1e": (np.asarray(inputs["ew_w1"], f32) * 32.0).astype(f8),
        "b1c": np.ascontiguousarray(np.asarray(inputs["cls_b1"], f32).reshape(MT1, 128).T),
        "b1w": np.ascontiguousarray(np.asarray(inputs["we_b1"], f32).reshape(MT1, 128).T),
        "b1e": np.ascontiguousarray(np.asarray(inputs["ew_b1"], f32).reshape(MT1, 128).T),
        "w2c": w2c.astype(f16),
        "w2w": np.asarray(inputs["we_w2"], f32),
        "w2e": w2e.astype(f8),
        "b2c": np.ascontiguousarray(b2c),
        "b2w": np.asarray(inputs["we_b2"], f32).reshape(E, 1),
        "b2e": np.asarray(inputs["ew_b2"], f32).reshape(E, 1),
        "colsel": colsel.astype(f16),
        "ident": ident,
        "iota": iota,
    }


def _per_core_inputs(xts, ksels, c):
    import ml_dtypes

    xt = xts[c]                                   # [D, BC] fp32
    xh = xt.astype(np.float16)
    xl = (xt - xh.astype(np.float32)).astype(np.float16)
    return {
        "xh": xh,
        "xl": xl,
        "x8": (xt * 4.0).astype(ml_dtypes.float8_e4m3),
        "ksel": ksels[c],
    }


def kernel(**inputs):
    x = np.asarray(inputs["x"], np.float32)
    rows_by_core, xts, ksels, sub_dir, sub_r = _host_prep(x, inputs["n_experts"])
    shared = _host_inputs(inputs)
    in_maps = [
        {**shared, **_per_core_inputs(xts, ksels, c)} for c in range(NCORES)
    ]

    nc = _get_nc(sub_dir, sub_r)

    from concourse.bass_utils import run_bass_kernel_spmd

    res = run_bass_kernel_spmd(nc, in_maps, core_ids=list(range(NCORES)))

    full = np.empty((B, C), np.float32)
    for c in range(NCORES):
        full[rows_by_core[c]] = res.results[c]["out"].T
    return full


if __name__ == "__main__":
    print("smoke build only")
    _get_nc(SUB_DIR, SUB_R)
    print("built ok")
